# revision 11
# baseline (speedup 1.0000x reference)
"""ConvexSH ColBERT loss kernel for 8 trn2 NeuronCores (v4).

Shards batch B=128 over 8 cores (16 rows each); host averages the 8 partial
sums. Doc layout per candidate n: one fully CONTIGUOUS 2 MB SWDGE cast-DMA
(f32->bf16) into [128, 32, 128] where partition p = b*8 + e covers tokens
k = e*32 + k'. Global k order is permuted - harmless under MaxSim's max.

Software-pipelined emission: iteration i emits square(i+1) | norm-chain(i) |
transposes+evac+matmul+max(i-1), so no engine's program order couples a
block's early stages to the previous block's late stages.

ssq = ACT Square -> two DVE pair-adds at 2x (TENSOR_REDUCE has no 2x mode,
tensor_tensor does) -> small 1x reduce. Normalize is one DVE tensor_tensor
with a pair-broadcast scale AP. Transposes/evacs go in 4 quarter-tiles with
a tunable ACT/DVE split. Label-only loss terms are precomputed on host; the
tail runs on flat [4,32] views with broadcast APs, and a manually emitted
InstLoadActFuncSet(natural_log_exp_and_others) after the last Sqrt hides
the tail's activation-table swap behind the final blocks.
"""

import sys
from contextlib import ExitStack

import numpy as np
import ml_dtypes

BF16NP = ml_dtypes.bfloat16

for _p in ("/opt/trn_rl_repo", "/root/.axon_site/_ro/trn_rl_repo"):
    if _p not in sys.path:
        sys.path.append(_p)

import concourse.bacc as bacc
import concourse.tile as tile
from concourse import mybir
from concourse.bass_utils import run_bass_kernel_spmd

AF = mybir.ActivationFunctionType
AX = mybir.AxisListType
ALU = mybir.AluOpType
F32 = mybir.dt.float32
BF16 = mybir.dt.bfloat16

NCORES = 8
B, LQ, LD, D, NWAY = 128, 32, 256, 128, 8
BS = B // NCORES  # 16 batch rows per core
NG = BS // 4      # 4 groups of 4 rows
NE = LD // 32     # 8 eighths of tokens per row -> partition p = b*8 + e
KT = 32           # tokens per partition (k')
ALPHA, GAMMA = 0.2, 2.0

EVAC_DVE_QUARTERS = 1   # how many of the 4 evac quarters go to DVE (rest ACT)
NLE_SET_ID = 6          # natural_log_exp_and_others in act_info.json

TRACE = False
LAST_RESULTS = None


def _build():
    nc = bacc.Bacc("TRN2", target_bir_lowering=False, detect_race_conditions=False)

    q_d = nc.dram_tensor("q", [128, NG, D], BF16, kind="ExternalInput")
    doc_d = nc.dram_tensor("doc", [NWAY, BS, LD, D], F32, kind="ExternalInput")
    mask_d = nc.dram_tensor("mask", [128, NWAY, KT], BF16, kind="ExternalInput")
    lab_d = nc.dram_tensor("lab", [BS, 6 * NWAY], F32, kind="ExternalInput")
    eye_d = nc.dram_tensor("eye", [128, 128], BF16, kind="ExternalInput")
    y_d = nc.dram_tensor("y", [1, 1], F32, kind="ExternalOutput")

    with tile.TileContext(nc) as tc, ExitStack() as ctx:
        singles = ctx.enter_context(tc.tile_pool(name="singles", bufs=1))
        dnp = ctx.enter_context(tc.tile_pool(name="dnp", bufs=6))
        sqp = ctx.enter_context(tc.tile_pool(name="sqp", bufs=3))
        dtp = ctx.enter_context(tc.tile_pool(name="dtp", bufs=2))
        smp = ctx.enter_context(tc.tile_pool(name="smp", bufs=3))
        psT = ctx.enter_context(tc.tile_pool(name="psT", bufs=4, space="PSUM"))
        psS = ctx.enter_context(tc.tile_pool(name="psS", bufs=2, space="PSUM"))

        # ---- constants -----------------------------------------------------
        sdum = singles.tile([1, 1], F32)
        nc.vector.memset(sdum, 1.0)
        nc.scalar.activation(out=sdum, in_=sdum, func=AF.Sqrt)

        eye_bf = singles.tile([128, 128], BF16)
        nc.sync.dma_start(out=eye_bf, in_=eye_d[:, :])

        blockones = singles.tile([128, NG], F32)
        nc.vector.memset(blockones, 0.0)
        for m in range(4):
            nc.vector.memset(blockones[m * 32:(m + 1) * 32, m:m + 1], 1.0)
        ones4 = singles.tile([4, 1], F32)
        nc.vector.memset(ones4, 1.0)

        # host-precomputed label constants: [t, a, b1, lnt, wts, w] x NWAY
        lab_sb = singles.tile([4, NG, 6 * NWAY], F32)
        nc.sync.dma_start(out=lab_sb, in_=lab_d.rearrange("(g m) c -> m g c", m=4))

        # masks in the (b, e) x (n, k') layout, cast to bf16
        mask_b = singles.tile([128, NWAY, KT], BF16)
        nc.sync.dma_start(out=mask_b, in_=mask_d[:, :, :])

        # ---- query path ----------------------------------------------------
        q_nat = singles.tile([128, NG, D], BF16)
        nc.sync.dma_start(out=q_nat, in_=q_d[:, :, :])

        ssq_q = singles.tile([128, NG], F32)
        qsq = singles.tile([128, D], BF16)
        invq = singles.tile([128, NG], F32)
        qT = singles.tile([128, NG, 128], BF16)

        def emit_q_path():
            for g in range(NG):
                nc.vector.scalar_tensor_tensor(
                    out=qsq, in0=q_nat[:, g, :], scalar=1.0, in1=q_nat[:, g, :],
                    op0=ALU.mult, op1=ALU.mult,
                    accum_out=ssq_q[:, g:g + 1])
            nc.scalar.activation(out=invq, in_=ssq_q, func=AF.Sqrt)
            nc.vector.reciprocal(invq, invq)
            ps_q = psT.tile([128, 8, 128], BF16, tag="psT")
            for g in range(NG):
                nc.tensor.transpose(ps_q[:, g, :], q_nat[:, g, :], eye_bf)
            nc.scalar.copy(qT.rearrange("p a b -> p (a b)"),
                           ps_q[:, 0:NG, :].rearrange("p a b -> p (a b)"))

        maxs = singles.tile([128, NG, NWAY], F32)

        # ---- software-pipelined main loop ---------------------------------
        state = {}

        def stage_dma(n, halves=False):
            dn = dnp.tile([128, KT, D], BF16, tag="dn", name=f"dn{n}")
            src_ap = doc_d[n].rearrange("b (e t) d -> (b e) (t d)", e=NE)
            if halves:
                H = KT // 2 * D
                nc.gpsimd.dma_start(out=dn.rearrange("p t d -> p (t d)")[:, 0:H],
                                    in_=src_ap[:, 0:H])
                nc.gpsimd.dma_start(out=dn.rearrange("p t d -> p (t d)")[:, H:2 * H],
                                    in_=src_ap[:, H:2 * H])
            else:
                nc.gpsimd.dma_start(out=dn.rearrange("p t d -> p (t d)"), in_=src_ap)
            state[n] = {"dn": dn}

        def stage_square(n, halves=False):
            sq = sqp.tile([128, KT, D], BF16, tag="sq", name=f"sq{n}")
            dnf = state[n]["dn"].rearrange("p t d -> p (t d)")
            sqf = sq.rearrange("p t d -> p (t d)")
            if halves:
                H = KT // 2 * D
                nc.scalar.activation(out=sqf[:, 0:H], in_=dnf[:, 0:H], func=AF.Square)
                nc.scalar.activation(out=sqf[:, H:2 * H], in_=dnf[:, H:2 * H],
                                     func=AF.Square)
            else:
                nc.scalar.activation(out=sqf, in_=dnf, func=AF.Square)
            state[n]["sq"] = sq

        def stage_norm(n):
            dn, sq = state[n]["dn"], state[n]["sq"]
            # two pair-add stages at DVE 2x, then a small 1x reduce
            nc.vector.tensor_add(sq[:, :, 0:64], sq[:, :, 0:64], sq[:, :, 64:128])
            nc.vector.tensor_add(sq[:, :, 0:32], sq[:, :, 0:32], sq[:, :, 32:64])
            nc.vector.tensor_add(sq[:, :, 0:16], sq[:, :, 0:16], sq[:, :, 16:32])
            ssq = smp.tile([128, KT], F32, tag="ssq", name=f"ssq{n}")
            nc.vector.reduce_sum(out=ssq, in_=sq[:, :, 0:16], axis=AX.X)
            rt = smp.tile([128, KT], F32, tag="rt", name=f"rt{n}")
            nc.vector.reciprocal_approx_fast(rt, ssq)
            nc.scalar.activation(out=rt, in_=rt, func=AF.Sqrt)  # 1/||d||
            scale2 = smp.tile([128, KT, 2], BF16, tag="scale2", name=f"s2{n}")
            nc.vector.tensor_mul(scale2[:, :, 0], rt, mask_b[:, n, :])
            nc.scalar.copy(scale2[:, :, 1], scale2[:, :, 0])
            dn4 = dn.rearrange("p t (h w) -> p t h w", w=2)
            nc.vector.tensor_tensor(
                out=dn4, in0=dn4,
                in1=scale2.unsqueeze(2).broadcast_to([128, KT, D // 2, 2]),
                op=ALU.mult)

        def stage_sim(n):
            dn = state[n]["dn"]
            dT = dtp.tile([128, KT, 128], BF16, tag="dT", name=f"dT{n}")
            for qt in range(4):
                ps = psT.tile([128, 8, 128], BF16, tag="psT", name=f"ps{n}_{qt}")
                for j in range(8):
                    nc.tensor.transpose(ps[:, j, :], dn[:, qt * 8 + j, :], eye_bf)
                quarter = dT[:, qt * 8:(qt + 1) * 8, :]
                if qt < 4 - EVAC_DVE_QUARTERS:
                    nc.scalar.copy(quarter.rearrange("p t d -> p (t d)"),
                                   ps.rearrange("p t d -> p (t d)"))
                else:
                    nc.vector.tensor_copy(quarter.rearrange("p t d -> p (t d)"),
                                          ps.rearrange("p t d -> p (t d)"))
            sim = psS.tile([128, NG, 256], F32, tag="sim", name=f"sim{n}")
            for g in range(NG):
                for m in range(4):
                    b = g * 4 + m
                    nc.tensor.matmul(sim[m * 32:(m + 1) * 32, g, :],
                                     lhsT=qT[:, g, m * 32:(m + 1) * 32],
                                     rhs=dT[:, :, NE * b:NE * (b + 1)],
                                     start=True, stop=True,
                                     tile_position=(0, m * 32))
            nc.vector.reduce_max(out=maxs[:, :, n], in_=sim, axis=AX.X)
            nc.vector.tensor_mul(maxs[:, :, n], maxs[:, :, n], invq)
            del state[n]

        stage_dma(0, halves=True)
        stage_dma(1)
        stage_square(0, halves=True)
        for i in range(NWAY):
            if i + 2 < NWAY:
                stage_dma(i + 2)
            if i + 1 < NWAY:
                stage_square(i + 1)
            stage_norm(i)
            if i == 0:
                emit_q_path()
            if i == NWAY - 1:
                # prefetch the tail's first (exp) table during the last blocks
                nc.scalar.activation(out=sdum, in_=sdum, func=AF.Exp)
            if i >= 1:
                stage_sim(i - 1)
        stage_sim(NWAY - 1)

        # ---- scores --------------------------------------------------------
        scores_ps = psT.tile([4, NG * NWAY], F32, tag="psT")
        nc.tensor.matmul(scores_ps, lhsT=blockones,
                         rhs=maxs.rearrange("p g n -> p (g n)"),
                         start=True, stop=True)
        sc = singles.tile([4, NG * NWAY], F32)  # [m, g*8+n]
        nc.vector.tensor_copy(sc, scores_ps)

        # ---- softmax over n (per g-slice); one Exp -------------------------
        sm = singles.tile([4, NG], F32)
        sc3 = sc.rearrange("p (g n) -> p g n", g=NG)
        nc.scalar.activation(out=sc, in_=sc, func=AF.Exp)
        nc.vector.reduce_sum(out=sm, in_=sc3, axis=AX.X)
        nc.vector.reciprocal(sm, sm)
        nc.vector.tensor_tensor(out=sc3, in0=sc3,
                                in1=sm.unsqueeze(2).broadcast_to([4, NG, NWAY]),
                                op=ALU.mult)

        # ---- ConvexSH loss (label-only terms precomputed on host) ---------
        F = NG * NWAY

        def fld(i):
            return lab_sb[:, :, i * NWAY:(i + 1) * NWAY]
        t3, a3, b13, lnt3, wts3, w3 = (fld(i) for i in range(6))

        def t32(name):
            t = singles.tile([4, F], F32, tag=name)
            return t, t.rearrange("p (g n) -> p g n", g=NG)

        # pom[:, 0, :] = p2, pom[:, 1, :] = 1 - p2; Ln and Exp run batched
        pom = singles.tile([4, 2, F], F32, tag="pom")
        pom4 = pom.rearrange("p a (g n) -> p a g n", g=NG)
        nc.vector.tensor_mul(pom4[:, 0], a3, sc3)
        nc.vector.tensor_add(pom4[:, 0], pom4[:, 0], b13)
        nc.vector.tensor_scalar(out=pom[:, 1, :], in0=pom[:, 0, :],
                                scalar1=-1.0, scalar2=1.0,
                                op0=ALU.mult, op1=ALU.add)
        pomf = pom.rearrange("p a f -> p (a f)")
        nc.scalar.activation(out=pomf, in_=pomf, func=AF.Ln)
        losses, losses3 = t32("losses")
        nc.vector.tensor_sub(losses3, lnt3, pom4[:, 0])
        nc.vector.tensor_mul(losses3, losses3, t3)
        nc.vector.tensor_tensor(
            out=pom4, in0=pom4,
            in1=wts3.unsqueeze(1).broadcast_to([4, 2, NG, NWAY]), op=ALU.mult)
        nc.scalar.activation(out=pomf, in_=pomf, func=AF.Exp)
        # pom[:, 0] = p2^wts, pom[:, 1] = (1-p2)^wts
        lv, lv3 = t32("lv")
        nc.vector.tensor_mul(lv3, w3, pom4[:, 1])
        t2, t23 = t32("t2")
        nc.vector.tensor_mul(t23, b13, pom4[:, 0])
        nc.vector.tensor_add(lv, lv, t2)
        nc.vector.tensor_mul(lv, lv, losses)

        partial = singles.tile([4, 1], F32)
        nc.vector.reduce_sum(out=partial, in_=lv, axis=AX.X)
        out_ps = psT.tile([1, 1], F32, tag="psT")
        nc.tensor.matmul(out_ps, lhsT=ones4, rhs=partial, start=True, stop=True)
        out_sb = singles.tile([1, 1], F32)
        nc.vector.tensor_copy(out_sb, out_ps)
        nc.sync.dma_start(out=y_d[:, :], in_=out_sb)

    nc.finalize()
    return nc


_nc_cache = None


def _q2(q):
    # [(m q), g, d] so the device upload is one contiguous 2 KB/partition DMA
    return np.ascontiguousarray(
        q.reshape(NG, 4, LQ, D).transpose(1, 2, 0, 3).reshape(128, NG, D))


def _m2(m):
    # [(b e), n, k'] matching the contiguous doc layout
    return np.ascontiguousarray(
        m.reshape(NWAY, BS, NE, KT).transpose(1, 2, 0, 3).reshape(128, NWAY, KT))


def _lab2(labels):
    t = labels[:, :NWAY].astype(np.float64)
    r = labels[:, NWAY:2 * NWAY].astype(np.float64)
    w = labels[:, 2 * NWAY:].astype(np.float64)
    a = 2.0 * w - 1.0
    b1 = 1.0 - w
    tinv = t * w + (1.0 - t) * (1.0 - w)
    lnt = np.log(tinv)
    rr = 1.0 / r
    wts = GAMMA - ALPHA * (rr - rr[:, :1])
    out = np.concatenate([t, a, b1, lnt, wts, w], axis=1)
    return np.ascontiguousarray(out, dtype=np.float32)


def kernel(query_reps, doc_reps, doc_masks, labels):
    global _nc_cache, LAST_RESULTS
    if _nc_cache is None:
        _nc_cache = _build()
    nc = _nc_cache

    eye = np.eye(128, dtype=BF16NP)
    labels = np.asarray(labels)
    in_maps = []
    for c in range(NCORES):
        sl = slice(c * BS, (c + 1) * BS)
        in_maps.append({
            "q": _q2(np.asarray(query_reps[sl], dtype=np.float32)).astype(BF16NP),
            "doc": np.ascontiguousarray(doc_reps[:, sl]).astype(np.float32, copy=False),
            "mask": _m2(np.asarray(doc_masks[:, sl], dtype=np.float32)).astype(BF16NP),
            "lab": _lab2(labels[sl]),
            "eye": eye,
        })

    kwargs = {}
    if TRACE:
        kwargs["trace"] = True
    res = run_bass_kernel_spmd(nc, in_maps, core_ids=list(range(NCORES)), **kwargs)
    LAST_RESULTS = res
    total = sum(float(res.results[c]["y"][0, 0]) for c in range(NCORES))
    return np.array(total / (B * NWAY), dtype=np.float32)
